# revision 7
# baseline (speedup 1.0000x reference)
"""Trainium2 Bass kernel for a 12-layer decoder-only transformer.

Sharding: 8-way tensor parallel (Megatron-style).
  - Attention: 2 heads per core (H=16 / 8 cores).
  - FFN: 512 of 4096 hidden per core (column/row split of W1/W2).
  - Tied embedding: vocab split 4000 per core for the output projection;
    full table replicated for the input gather.
  - Activations replicated; 2 AllReduces per layer with the residual
    (x/8 per core) and biases (b/8) folded into the reduced payload.

Layout: the residual stream lives TRANSPOSED on chip, xT = [D, L] as
8 SBUF tiles of [128, 1024].  Every matmul then takes natural operands
with zero per-layer transposes:
  - qT/kT [hd, L] = W.T x.T         (lhsT = W tile, rhs = xT)
  - ST [keys, q]  = (kT-slice).T qT  -> exp on scalar engine
  - OT' [1+64, q] = V'[keys, 1+64].T PT   (row 0 = softmax sums, via
                     ones-augmented V; divide via rank-1 broadcast matmul)
  - attnT [D, L]  = (Wo-slice).T oT
  - hT [FF, L]    = (W1-slice).T xT ; fT [D, L] = (W2-slice).T hT
  - logits [L, VS] = xT-slice.T embT  (+ final_b via ones-row matmul)
LayerNorm runs along partitions (D) using ones-column matmuls for the
stats and rank-1 matmul broadcasts for mu/rstd.

PSUM budget (8 banks = 16KB/partition) via three tags:
  "mm" [128,1024]f32 x2 bufs (8KB), "acc" x1 (4KB), "row" [1,1024] x1 (4KB).
"""

import sys

sys.path.insert(0, "/opt/trn_rl_repo")

import contextlib
import math

import numpy as np
import ml_dtypes

import concourse.bass as bass
import concourse.mybir as mybir
import concourse.tile as tile
from concourse import bacc
from concourse.bass_utils import run_bass_kernel_spmd
from concourse.masks import make_identity, make_upper_triangular

F32 = mybir.dt.float32
BF16 = mybir.dt.bfloat16
I32 = mybir.dt.int32
AF = mybir.ActivationFunctionType

L, D, H, DK, DV, FF, V, NLAYERS = 1024, 1024, 16, 64, 64, 4096, 32000, 12
NC = 8          # cores
P = 128         # partitions
HC = H // NC    # heads per core (2)
HVS = HC * DV   # per-core head-concat width (128)
FS = FF // NC   # FF shard (512)
VS = V // NC    # vocab shard (4000)
LT = L // P     # L tiles (8)
DT = D // P     # D tiles (8)
FT = FS // P    # FF shard tiles (4)
EMB_SCALE = 2.0 * math.sqrt(3.0 * D)
SCORE_SCALE = 1.0 / math.sqrt(float(D))
EPS = 1e-5

NSL = [(0, 512), (512, 512)]  # 512-col slices of [0, 1024)


def _nslices(start, end, step=512):
    out = []
    a = start
    while a < end:
        w = min(step, end - a)
        out.append((a, w))
        a += w
    return out


def build_program(nlayers=NLAYERS):
    nc = bacc.Bacc(num_devices=NC)

    emb_full = nc.dram_tensor("emb_full", [V, D], F32, kind="ExternalInput")
    tok = nc.dram_tensor("tok", [LT, P, 1], I32, kind="ExternalInput")
    pe_d = nc.dram_tensor("pe", [LT, P, D], F32, kind="ExternalInput")
    wq_d = nc.dram_tensor("wq", [nlayers, DT, P, HVS], BF16, kind="ExternalInput")
    wk_d = nc.dram_tensor("wk", [nlayers, DT, P, HVS], BF16, kind="ExternalInput")
    wv_d = nc.dram_tensor("wv", [nlayers, DT, P, HVS], BF16, kind="ExternalInput")
    wo_d = nc.dram_tensor("wo", [nlayers, DT, P, P], BF16, kind="ExternalInput")
    w1_d = nc.dram_tensor("w1", [nlayers, DT, FT, P, P], BF16, kind="ExternalInput")
    w2_d = nc.dram_tensor("w2", [nlayers, FT, DT, P, P], BF16, kind="ExternalInput")
    b1_d = nc.dram_tensor("b1c", [nlayers, FT, P, 1], F32, kind="ExternalInput")
    b2_d = nc.dram_tensor("b2c", [nlayers, DT, P, 1], F32, kind="ExternalInput")
    lns_d = nc.dram_tensor("lns", [nlayers, 2, DT, P, 1], F32, kind="ExternalInput")
    lnb_d = nc.dram_tensor("lnb", [nlayers, 2, DT, P, 1], F32, kind="ExternalInput")
    embt_d = nc.dram_tensor("embt", [DT, P, VS], BF16, kind="ExternalInput")
    fb_d = nc.dram_tensor("fb", [1, VS], BF16, kind="ExternalInput")
    logits_d = nc.dram_tensor("logits", [L, VS], F32, kind="ExternalOutput")

    rg = [list(range(NC))]

    with tile.TileContext(nc) as tc, contextlib.ExitStack() as ctx:
        singles = ctx.enter_context(tc.tile_pool(name="singles", bufs=1))
        persist = ctx.enter_context(tc.tile_pool(name="persist", bufs=1))
        wpool = ctx.enter_context(tc.tile_pool(name="wpool", bufs=2))
        work = ctx.enter_context(tc.tile_pool(name="work", bufs=3))
        vpool = ctx.enter_context(tc.tile_pool(name="vpool", bufs=1))
        psum = ctx.enter_context(tc.tile_pool(name="psum", bufs=1, space="PSUM"))
        dram = ctx.enter_context(tc.tile_pool(name="dram", bufs=2, space="DRAM"))

        def ps_mm(name):
            return psum.tile([P, L], F32, name=name, tag="mm", bufs=2)

        # ---------------- constants ----------------
        ident = singles.tile([P, P], BF16, name="ident")
        make_identity(nc, ident)
        tri01 = singles.tile([P, P], BF16, name="tri01")
        # tri01[r, c] = 1 if r <= c else 0  (valid keys r for query c)
        make_upper_triangular(nc, tri01, val=1.0, diag=True)
        ones_col = singles.tile([P, 1], F32, name="ones_col")
        nc.vector.memset(ones_col, 1.0)
        ones_row64 = singles.tile([1, 64], F32, name="ones_row64")
        nc.vector.memset(ones_row64, 1.0)
        ones_row128 = singles.tile([1, P], F32, name="ones_row128")
        nc.vector.memset(ones_row128, 1.0)
        ones_rowP_bf = singles.tile([1, P], BF16, name="ones_rowP_bf")
        nc.vector.memset(ones_rowP_bf, 1.0)
        fb_s = singles.tile([1, VS], BF16, name="fb_s")
        nc.sync.dma_start(out=fb_s, in_=fb_d[:, :])

        # persistent activations
        xbf = [persist.tile([P, L], BF16, name=f"xbf{d}") for d in range(DT)]
        vres = [vpool.tile([P, L], F32, name=f"vres{d}") for d in range(DT)]

        def layernorm(which, lns_s, lnb_s):
            """xbf[dt] = ((vres - mu) * rstd) * s + b, stats along D."""
            # pass 1: sums
            sum_ps = psum.tile([1, L], F32, name="sum_ps", tag="row", bufs=1)
            for dt in range(DT):
                for (n0, nw) in NSL:
                    nc.tensor.matmul(
                        sum_ps[:, n0:n0 + nw], ones_col, vres[dt][:, n0:n0 + nw],
                        start=(dt == 0), stop=(dt == DT - 1), skip_group_check=True,
                    )
            m_row = work.tile([1, L], F32, name="m_row", tag="rows", bufs=3)
            nc.scalar.activation(m_row, sum_ps, AF.Copy, scale=1.0 / D)
            # pass 2: sum of squares (+ eps folded into the mean-square)
            sq_ps = psum.tile([1, L], F32, name="sq_ps", tag="row", bufs=1)
            for dt in range(DT):
                vsq = work.tile([P, L], F32, name="vsq", tag="f32a", bufs=2)
                nc.scalar.activation(vsq, vres[dt], AF.Square)
                for (n0, nw) in NSL:
                    nc.tensor.matmul(
                        sq_ps[:, n0:n0 + nw], ones_col, vsq[:, n0:n0 + nw],
                        start=(dt == 0), stop=(dt == DT - 1), skip_group_check=True,
                    )
            msq_row = work.tile([1, L], F32, name="msq_row", tag="rows", bufs=3)
            nc.scalar.activation(msq_row, sq_ps, AF.Copy, scale=1.0 / D, bias=EPS)
            var_row = work.tile([1, L], F32, name="var_row", tag="rows", bufs=3)
            nc.vector.tensor_mul(var_row, m_row, m_row)
            nc.vector.tensor_sub(var_row, msq_row, var_row)  # E[x^2]+eps - mu^2
            std_row = work.tile([1, L], F32, name="std_row", tag="rows", bufs=3)
            nc.scalar.activation(std_row, var_row, AF.Sqrt)
            a_row = work.tile([1, L], F32, name="a_row", tag="rows", bufs=3)
            nc.vector.reciprocal(a_row, std_row)
            c_row = work.tile([1, L], F32, name="c_row", tag="rows", bufs=3)
            nc.vector.tensor_mul(c_row, m_row, a_row)
            nc.vector.tensor_scalar_mul(c_row, c_row, -1.0)
            # broadcast rstd and -mu*rstd to [128, L]
            a_ps = ps_mm("a_ps")
            c_ps = ps_mm("c_ps")
            for (n0, nw) in NSL:
                nc.tensor.matmul(a_ps[:, n0:n0 + nw], ones_row128,
                                 a_row[:, n0:n0 + nw], start=True, stop=True,
                                 skip_group_check=True)
                nc.tensor.matmul(c_ps[:, n0:n0 + nw], ones_row128,
                                 c_row[:, n0:n0 + nw], start=True, stop=True,
                                 skip_group_check=True)
            for dt in range(DT):
                t = work.tile([P, L], F32, name="lnt", tag="f32b", bufs=2)
                nc.vector.tensor_mul(t, vres[dt], a_ps)
                nc.vector.tensor_add(t, t, c_ps)
                nc.scalar.activation(
                    xbf[dt], t, AF.Identity,
                    bias=lnb_s[:, which * DT + dt:which * DT + dt + 1],
                    scale=lns_s[:, which * DT + dt:which * DT + dt + 1],
                )

        # =========== embedding: gather + scale + pe, then transpose ===========
        x0bf = []
        for lt in range(LT):
            idx_t = work.tile([P, 1], I32, name="idx", tag="idx")
            nc.sync.dma_start(out=idx_t, in_=tok[lt])
            g = work.tile([P, D], F32, name="gath", tag="f32a", bufs=2)
            nc.gpsimd.indirect_dma_start(
                out=g,
                out_offset=None,
                in_=emb_full[:, :],
                in_offset=bass.IndirectOffsetOnAxis(ap=idx_t[:, :1], axis=0),
            )
            pe_t = work.tile([P, D], F32, name="pet", tag="f32b", bufs=2)
            nc.sync.dma_start(out=pe_t, in_=pe_d[lt])
            x0 = work.tile([P, D], F32, name="x0", tag="f32a", bufs=2)
            nc.scalar.activation(x0, g, AF.Copy, scale=EMB_SCALE)
            nc.vector.tensor_add(x0, x0, pe_t)
            xb = work.tile([P, D], BF16, name=f"x0bf{lt}", tag="x0bf", bufs=8)
            nc.scalar.activation(xb, x0, AF.Copy)
            x0bf.append(xb)
        for dt in range(DT):
            for lt in range(LT):
                tp = psum.tile([P, P], BF16, name="tp", tag="mm", bufs=2)
                nc.tensor.transpose(tp, x0bf[lt][:, dt * P:(dt + 1) * P], ident)
                nc.vector.tensor_copy(xbf[dt][:, lt * P:(lt + 1) * P], tp)

        # ================= layers =================
        for ly in range(nlayers):
            wq_s = wpool.tile([P, DT, HVS], BF16, name="wq_s", tag="wq")
            nc.sync.dma_start(out=wq_s, in_=wq_d[ly].rearrange("k p m -> p k m"))
            wk_s = wpool.tile([P, DT, HVS], BF16, name="wk_s", tag="wk")
            nc.sync.dma_start(out=wk_s, in_=wk_d[ly].rearrange("k p m -> p k m"))
            wv_s = wpool.tile([P, DT, HVS], BF16, name="wv_s", tag="wv")
            nc.sync.dma_start(out=wv_s, in_=wv_d[ly].rearrange("k p m -> p k m"))
            wo_s = wpool.tile([P, DT, P], BF16, name="wo_s", tag="wo")
            nc.sync.dma_start(out=wo_s, in_=wo_d[ly].rearrange("k p m -> p k m"))
            w1_s = wpool.tile([P, DT, FT, P], BF16, name="w1_s", tag="w1")
            nc.sync.dma_start(out=w1_s, in_=w1_d[ly].rearrange("k f p m -> p k f m"))
            w2_s = wpool.tile([P, FT, DT, P], BF16, name="w2_s", tag="w2")
            nc.sync.dma_start(out=w2_s, in_=w2_d[ly].rearrange("f k p m -> p f k m"))
            b1_s = wpool.tile([P, FT], F32, name="b1_s", tag="b1")
            nc.sync.dma_start(out=b1_s, in_=b1_d[ly].rearrange("f p o -> p (f o)"))
            b2_s = wpool.tile([P, DT], F32, name="b2_s", tag="b2")
            nc.sync.dma_start(out=b2_s, in_=b2_d[ly].rearrange("d p o -> p (d o)"))
            lns_s = wpool.tile([P, 2 * DT], F32, name="lns_s", tag="lns")
            nc.sync.dma_start(out=lns_s, in_=lns_d[ly].rearrange("i d p o -> p (i d o)"))
            lnb_s = wpool.tile([P, 2 * DT], F32, name="lnb_s", tag="lnb")
            nc.sync.dma_start(out=lnb_s, in_=lnb_d[ly].rearrange("i d p o -> p (i d o)"))

            # ---- QKV projections ----
            def proj(w_s, outname):
                ps = ps_mm("proj_ps")
                for kt in range(DT):
                    for (n0, nw) in NSL:
                        nc.tensor.matmul(
                            ps[:, n0:n0 + nw],
                            w_s[:, kt, :],
                            xbf[kt][:, n0:n0 + nw],
                            start=(kt == 0),
                            stop=(kt == DT - 1),
                            skip_group_check=True,
                        )
                ot = work.tile([P, L], BF16, name=outname, tag=outname,
                               bufs=(1 if outname == "vT" else 2))
                nc.scalar.activation(ot, ps, AF.Copy)
                return ot

            qT = proj(wq_s, "qT")
            kT = proj(wk_s, "kT")
            vT = proj(wv_s, "vT")

            # V natural [keys, 130]: [ones, v_h0 (64), ones, v_h1 (64)]
            vnat = []
            for kt in range(LT):
                tp = psum.tile([P, P], BF16, name="vtp", tag="mm", bufs=2)
                nc.tensor.transpose(tp, vT[:, kt * P:(kt + 1) * P], ident)
                vn = work.tile([P, 130], BF16, name="vnat", tag="vnat", bufs=8)
                nc.vector.memset(vn[:, 64:65], 1.0)
                nc.vector.memset(vn[:, 129:130], 1.0)
                nc.vector.tensor_copy(vn[:, 0:64], tp[:, 0:64])
                nc.vector.tensor_copy(vn[:, 65:129], tp[:, 64:128])
                vnat.append(vn)

            # ---- attention per head ----
            oT = work.tile([P, L], BF16, name="oT", tag="oT", bufs=2)
            for h in range(HC):
                hp = 64 * h
                ot_ps = psum.tile([P, L], F32, name="ot_ps", tag="acc", bufs=1)
                for kt in range(LT):
                    q0 = kt * P
                    st_ps = ps_mm("st_ps")
                    for (n0, nw) in _nslices(q0, L):
                        nc.tensor.matmul(
                            st_ps[:, n0:n0 + nw],
                            kT[hp:hp + 64, q0:q0 + P],
                            qT[hp:hp + 64, n0:n0 + nw],
                            start=True, stop=True, skip_group_check=True,
                        )
                    pt = work.tile([P, L], BF16, name="pt", tag="pt", bufs=3)
                    nc.scalar.activation(
                        pt[:, q0:L], st_ps[:, q0:L], AF.Exp, scale=SCORE_SCALE
                    )
                    nc.vector.tensor_mul(pt[:, q0:q0 + P], pt[:, q0:q0 + P], tri01)
                    for (n0, nw0) in NSL:
                        a = max(n0, q0)
                        w = n0 + nw0 - a
                        if w <= 0:
                            continue
                        last_kt = min(LT - 1, (n0 + nw0 - 1) // P)
                        nc.tensor.matmul(
                            ot_ps[0:65, a:a + w],
                            vnat[kt][:, 65 * h:65 * h + 65],
                            pt[:, a:a + w],
                            start=(kt == 0),
                            stop=(kt == last_kt),
                            skip_group_check=True,
                        )
                # normalize rows 1:65 by row 0 (softmax sums)
                s_row = work.tile([1, L], F32, name="s_row", tag="rows", bufs=3)
                nc.scalar.activation(s_row, ot_ps[64:65, :], AF.Copy)
                r_row = work.tile([1, L], F32, name="r_row", tag="rows", bufs=3)
                nc.vector.reciprocal(r_row, s_row)
                bc_ps = psum.tile([64, L], F32, name="bc_ps", tag="mm", bufs=2)
                for (n0, nw) in NSL:
                    nc.tensor.matmul(
                        bc_ps[:, n0:n0 + nw], ones_row64, r_row[:, n0:n0 + nw],
                        start=True, stop=True, skip_group_check=True,
                    )
                bc_sb = work.tile([64, L], F32, name="bc_sb", tag="f32a", bufs=2)
                nc.scalar.activation(bc_sb, bc_ps, AF.Copy)
                nc.vector.tensor_mul(oT[hp:hp + 64, :], ot_ps[0:64, :], bc_sb)

            # ---- Wo + AllReduce (residual x/8 folded) ----
            cc_in = dram.tile([P, DT * L], F32, name="cc_in", tag="cc_in")
            cc_out = dram.tile(
                [P, DT * L], F32, name="cc_out", tag="cc_out", addr_space="Shared"
            )
            for dt in range(DT):
                wo_ps = ps_mm("wo_ps")
                for (n0, nw) in NSL:
                    nc.tensor.matmul(
                        wo_ps[:, n0:n0 + nw], wo_s[:, dt, :], oT[:, n0:n0 + nw],
                        start=True, stop=True, skip_group_check=True,
                    )
                t1 = work.tile([P, L], F32, name="t1", tag="f32a", bufs=2)
                nc.scalar.activation(t1, xbf[dt], AF.Copy, scale=1.0 / NC)
                t2 = work.tile([P, L], F32, name="t2", tag="f32b", bufs=2)
                nc.vector.tensor_add(t2, t1, wo_ps)
                nc.sync.dma_start(out=cc_in[:, dt * L:(dt + 1) * L], in_=t2)
            nc.gpsimd.collective_compute(
                "AllReduce", mybir.AluOpType.add, replica_groups=rg,
                ins=[cc_in[:, :].opt()], outs=[cc_out[:, :].opt()],
            )
            for dt in range(DT):
                nc.sync.dma_start(out=vres[dt], in_=cc_out[:, dt * L:(dt + 1) * L])

            layernorm(0, lns_s, lnb_s)

            # ---- FFN1 ----
            hT = []
            for mt in range(FT):
                h_ps = ps_mm("h_ps")
                for kt in range(DT):
                    for (n0, nw) in NSL:
                        nc.tensor.matmul(
                            h_ps[:, n0:n0 + nw], w1_s[:, kt, mt, :],
                            xbf[kt][:, n0:n0 + nw],
                            start=(kt == 0), stop=(kt == DT - 1),
                            skip_group_check=True,
                        )
                ht = work.tile([P, L], BF16, name="hT", tag=f"hT{mt}", bufs=1)
                nc.scalar.activation(ht, h_ps, AF.Relu, bias=b1_s[:, mt:mt + 1])
                hT.append(ht)

            # ---- FFN2 + AllReduce (residual x/8 and b2/8 folded) ----
            cc2_in = dram.tile([P, DT * L], F32, name="cc2_in", tag="cc_in")
            cc2_out = dram.tile(
                [P, DT * L], F32, name="cc2_out", tag="cc_out", addr_space="Shared"
            )
            for dt in range(DT):
                f_ps = ps_mm("f_ps")
                for kt in range(FT):
                    for (n0, nw) in NSL:
                        nc.tensor.matmul(
                            f_ps[:, n0:n0 + nw], w2_s[:, kt, dt, :],
                            hT[kt][:, n0:n0 + nw],
                            start=(kt == 0), stop=(kt == FT - 1),
                            skip_group_check=True,
                        )
                t1 = work.tile([P, L], F32, name="t1b", tag="f32a", bufs=2)
                nc.scalar.activation(
                    t1, xbf[dt], AF.Identity, bias=b2_s[:, dt:dt + 1], scale=1.0 / NC
                )
                t2 = work.tile([P, L], F32, name="t2b", tag="f32b", bufs=2)
                nc.vector.tensor_add(t2, t1, f_ps)
                nc.sync.dma_start(out=cc2_in[:, dt * L:(dt + 1) * L], in_=t2)
            nc.gpsimd.collective_compute(
                "AllReduce", mybir.AluOpType.add, replica_groups=rg,
                ins=[cc2_in[:, :].opt()], outs=[cc2_out[:, :].opt()],
            )
            for dt in range(DT):
                nc.sync.dma_start(out=vres[dt], in_=cc2_out[:, dt * L:(dt + 1) * L])

            layernorm(1, lns_s, lnb_s)

        # ================= final logits =================
        for (v0, vw) in _nslices(0, VS):
            ets = []
            for kt in range(DT):
                et = work.tile([P, 512], BF16, name="et", tag="et", bufs=8)
                nc.sync.dma_start(out=et[:, 0:vw], in_=embt_d[kt][:, v0:v0 + vw])
                ets.append(et)
            for lt in range(LT):
                lg_ps = psum.tile([P, 512], F32, name="lg_ps", tag="mm", bufs=2)
                for kt in range(DT):
                    nc.tensor.matmul(
                        lg_ps[:, 0:vw],
                        xbf[kt][:, lt * P:(lt + 1) * P],
                        ets[kt][:, 0:vw],
                        start=(kt == 0), stop=False, skip_group_check=True,
                    )
                nc.tensor.matmul(
                    lg_ps[:, 0:vw], ones_rowP_bf, fb_s[:, v0:v0 + vw],
                    start=False, stop=True, skip_group_check=True,
                )
                lg = work.tile([P, 512], F32, name="lg", tag="lg", bufs=2)
                nc.scalar.activation(lg[:, 0:vw], lg_ps[:, 0:vw], AF.Copy)
                nc.sync.dma_start(
                    out=logits_d[lt * P:(lt + 1) * P, v0:v0 + vw], in_=lg[:, 0:vw]
                )

    nc.finalize()
    return nc


# ====================== host side ======================

def _pe_host():
    i = 2.0 * np.arange(D // 2, dtype=np.float32)
    base = np.float32(10000.0) ** (-i / np.float32(D))
    half = np.arange(L, dtype=np.float32)[:, None] * base[None, :]
    return np.stack([np.sin(half), np.cos(half)], axis=-1).reshape(L, D).astype(np.float32)


def prep_inputs(inputs, nlayers=NLAYERS):
    bf = ml_dtypes.bfloat16
    tokens = np.asarray(inputs["tokens"]).reshape(L).astype(np.int32)
    emb = np.ascontiguousarray(np.asarray(inputs["emb"], dtype=np.float32))
    tok_t = np.ascontiguousarray(tokens.reshape(LT, P, 1))
    pe_t = np.ascontiguousarray(_pe_host().reshape(LT, P, D))

    Wq = np.asarray(inputs["Wq"], dtype=np.float32)[:nlayers]
    Wk = np.asarray(inputs["Wk"], dtype=np.float32)[:nlayers]
    Wv = np.asarray(inputs["Wv"], dtype=np.float32)[:nlayers]
    Wo = np.asarray(inputs["Wo"], dtype=np.float32)[:nlayers]
    W1 = np.asarray(inputs["W1"], dtype=np.float32)[:nlayers]
    W2 = np.asarray(inputs["W2"], dtype=np.float32)[:nlayers]
    b1 = np.asarray(inputs["b1"], dtype=np.float32)[:nlayers]
    b2 = np.asarray(inputs["b2"], dtype=np.float32)[:nlayers]
    l1s = np.asarray(inputs["ln1_s"], dtype=np.float32)[:nlayers]
    l1b = np.asarray(inputs["ln1_b"], dtype=np.float32)[:nlayers]
    l2s = np.asarray(inputs["ln2_s"], dtype=np.float32)[:nlayers]
    l2b = np.asarray(inputs["ln2_b"], dtype=np.float32)[:nlayers]
    fbv = np.asarray(inputs["final_b"], dtype=np.float32)

    def C(a, dtype=None):
        a = np.ascontiguousarray(a)
        return a.astype(dtype) if dtype is not None else a

    maps = []
    for c in range(NC):
        hs = slice(HC * c, HC * (c + 1))
        wq = Wq[:, hs].transpose(0, 2, 1, 3).reshape(nlayers, D, HVS)
        wk = Wk[:, hs].transpose(0, 2, 1, 3).reshape(nlayers, D, HVS)
        wv = Wv[:, hs].transpose(0, 2, 1, 3).reshape(nlayers, D, HVS)
        wo = Wo[:, P * c:P * (c + 1), :]        # [nl, 128(HV), D]
        w1 = W1[:, :, FS * c:FS * (c + 1)]      # [nl, D, FS]
        w2 = W2[:, FS * c:FS * (c + 1), :]      # [nl, FS, D]
        m = {
            "emb_full": emb,
            "tok": tok_t,
            "pe": pe_t,
            "wq": C(wq.reshape(nlayers, DT, P, HVS), bf),
            "wk": C(wk.reshape(nlayers, DT, P, HVS), bf),
            "wv": C(wv.reshape(nlayers, DT, P, HVS), bf),
            # wo_d[ly, dt] = Wo_c[:, 128dt:128dt+128]  ([HV=128, 128])
            "wo": C(wo.reshape(nlayers, P, DT, P).transpose(0, 2, 1, 3), bf),
            # w1_d[ly, kt, mt] = W1_c[128kt:.., 128mt:..]
            "w1": C(w1.reshape(nlayers, DT, P, FT, P).transpose(0, 1, 3, 2, 4), bf),
            # w2_d[ly, kt, dt] = W2_c[128kt:.., 128dt:..]
            "w2": C(w2.reshape(nlayers, FT, P, DT, P).transpose(0, 1, 3, 2, 4), bf),
            "b1c": C(b1[:, FS * c:FS * (c + 1)].reshape(nlayers, FT, P, 1)),
            "b2c": C((b2 / NC).reshape(nlayers, DT, P, 1)),
            "lns": C(np.stack([l1s, l2s], axis=1).reshape(nlayers, 2, DT, P, 1)),
            "lnb": C(np.stack([l1b, l2b], axis=1).reshape(nlayers, 2, DT, P, 1)),
            "embt": C(emb[VS * c:VS * (c + 1), :].T.reshape(DT, P, VS), bf),
            "fb": C(fbv[VS * c:VS * (c + 1)].reshape(1, VS), bf),
        }
        maps.append(m)
    return maps


_PROGRAM = None


def kernel(**inputs):
    global _PROGRAM
    if _PROGRAM is None:
        _PROGRAM = build_program(NLAYERS)
    maps = prep_inputs(inputs, NLAYERS)
    res = run_bass_kernel_spmd(_PROGRAM, maps, core_ids=list(range(NC)), trace=False)
    logits = np.concatenate(
        [res.results[c]["logits"] for c in range(NC)], axis=1
    ).astype(np.float32)
    return logits.reshape(1, L, V)


if __name__ == "__main__":
    nc = build_program(int(sys.argv[1]) if len(sys.argv) > 1 else 1)
    print("build OK; instructions:", len(nc.inst_map))


# revision 11
# speedup vs baseline: 5.5540x; 5.5540x over previous
"""Trainium2 Bass kernel for a 12-layer decoder-only transformer.

Sharding: 8-way tensor parallel (Megatron-style).
  - Attention: 2 heads per core (H=16 / 8 cores).
  - FFN: 512 of 4096 hidden per core (column/row split of W1/W2).
  - Tied embedding: vocab split 4000 per core for the output projection;
    full table replicated for the input gather.
  - Activations replicated; 2 AllReduces per layer with the residual
    (x/8 per core) and biases (b/8) folded into the reduced payload.

Layout: the residual stream lives TRANSPOSED on chip, xT = [D, L] as
8 SBUF tiles of [128, 1024].  Every matmul then takes natural operands
with zero per-layer transposes:
  - qT/kT [hd, L] = W.T x.T         (lhsT = W tile, rhs = xT)
  - ST [keys, q]  = (kT-slice).T qT  -> exp on scalar engine
  - OT' [1+64, q] = V'[keys, 1+64].T PT   (row 0 = softmax sums, via
                     ones-augmented V; divide via rank-1 broadcast matmul)
  - attnT [D, L]  = (Wo-slice).T oT
  - hT [FF, L]    = (W1-slice).T xT ; fT [D, L] = (W2-slice).T hT
  - logits [L, VS] = xT-slice.T embT  (+ final_b via ones-row matmul)
LayerNorm runs along partitions (D) using ones-column matmuls for the
stats and rank-1 matmul broadcasts for mu/rstd.

PSUM budget (8 banks = 16KB/partition) via three tags:
  "mm" [128,1024]f32 x2 bufs (8KB), "acc" x1 (4KB), "row" [1,1024] x1 (4KB).
"""

import sys

sys.path.insert(0, "/opt/trn_rl_repo")

import contextlib
import math

import numpy as np
import ml_dtypes

import concourse.bass as bass
import concourse.mybir as mybir
import concourse.tile as tile
from concourse import bacc
from concourse.bass_utils import run_bass_kernel_spmd
from concourse.masks import make_identity, make_upper_triangular

F32 = mybir.dt.float32
BF16 = mybir.dt.bfloat16
I32 = mybir.dt.int32
AF = mybir.ActivationFunctionType

L, D, H, DK, DV, FF, V, NLAYERS = 1024, 1024, 16, 64, 64, 4096, 32000, 12
NC = 8          # cores
P = 128         # partitions
HC = H // NC    # heads per core (2)
HVS = HC * DV   # per-core head-concat width (128)
FS = FF // NC   # FF shard (512)
VS = V // NC    # vocab shard (4000)
LT = L // P     # L tiles (8)
DT = D // P     # D tiles (8)
FT = FS // P    # FF shard tiles (4)
EMB_SCALE = 2.0 * math.sqrt(3.0 * D)
SCORE_SCALE = 1.0 / math.sqrt(float(D))
EPS = 1e-5

NSL = [(0, 512), (512, 512)]  # 512-col slices of [0, 1024)


def _nslices(start, end, step=512):
    out = []
    a = start
    while a < end:
        w = min(step, end - a)
        out.append((a, w))
        a += w
    return out


def build_program(nlayers=NLAYERS):
    nc = bacc.Bacc(num_devices=NC)

    emb_full = nc.dram_tensor("emb_full", [V, D], F32, kind="ExternalInput")
    tok = nc.dram_tensor("tok", [LT, P, 1], I32, kind="ExternalInput")
    pe_d = nc.dram_tensor("pe", [LT, P, D], F32, kind="ExternalInput")
    wq_d = nc.dram_tensor("wq", [nlayers, DT, P, HVS], BF16, kind="ExternalInput")
    wk_d = nc.dram_tensor("wk", [nlayers, DT, P, HVS], BF16, kind="ExternalInput")
    wv_d = nc.dram_tensor("wv", [nlayers, DT, P, HVS], BF16, kind="ExternalInput")
    wo_d = nc.dram_tensor("wo", [nlayers, DT, P, P], BF16, kind="ExternalInput")
    w1_d = nc.dram_tensor("w1", [nlayers, DT, FT, P, P], BF16, kind="ExternalInput")
    w2_d = nc.dram_tensor("w2", [nlayers, FT, DT, P, P], BF16, kind="ExternalInput")
    b1_d = nc.dram_tensor("b1c", [nlayers, FT, P, 1], F32, kind="ExternalInput")
    b2_d = nc.dram_tensor("b2c", [nlayers, DT, P, 1], F32, kind="ExternalInput")
    lns_d = nc.dram_tensor("lns", [nlayers, 2, DT, P, 1], F32, kind="ExternalInput")
    lnb_d = nc.dram_tensor("lnb", [nlayers, 2, DT, P, 1], F32, kind="ExternalInput")
    embt_d = nc.dram_tensor("embt", [DT, P, VS], BF16, kind="ExternalInput")
    fb_d = nc.dram_tensor("fb", [1, VS], BF16, kind="ExternalInput")
    logits_d = nc.dram_tensor("logits", [L, VS], F32, kind="ExternalOutput")

    rg = [list(range(NC))]

    with tile.TileContext(nc) as tc, contextlib.ExitStack() as ctx:
        singles = ctx.enter_context(tc.tile_pool(name="singles", bufs=1))
        persist = ctx.enter_context(tc.tile_pool(name="persist", bufs=1))
        wpool = ctx.enter_context(tc.tile_pool(name="wpool", bufs=2))
        work = ctx.enter_context(tc.tile_pool(name="work", bufs=3))
        vpool = ctx.enter_context(tc.tile_pool(name="vpool", bufs=1))
        psum = ctx.enter_context(tc.tile_pool(name="psum", bufs=1, space="PSUM"))
        dram = ctx.enter_context(tc.tile_pool(name="dram", bufs=2, space="DRAM"))

        def ps_mm(name):
            return psum.tile([P, L], F32, name=name, tag="mm", bufs=2)

        # ---------------- constants ----------------
        ident = singles.tile([P, P], BF16, name="ident")
        make_identity(nc, ident)
        tri01 = singles.tile([P, P], BF16, name="tri01")
        # tri01[r, c] = 1 if r <= c else 0  (valid keys r for query c)
        make_upper_triangular(nc, tri01, val=1.0, diag=True)
        ones_col = singles.tile([P, 1], BF16, name="ones_col")
        nc.vector.memset(ones_col, 1.0)
        ones_row64 = singles.tile([1, 64], F32, name="ones_row64")
        nc.vector.memset(ones_row64, 1.0)
        ones_row128 = singles.tile([1, P], F32, name="ones_row128")
        nc.vector.memset(ones_row128, 1.0)
        ones_rowP_bf = singles.tile([1, P], BF16, name="ones_rowP_bf")
        nc.vector.memset(ones_rowP_bf, 1.0)
        fb_s = singles.tile([1, VS], BF16, name="fb_s")
        nc.sync.dma_start(out=fb_s, in_=fb_d[:, :])

        # persistent activations
        xbf = [persist.tile([P, L], BF16, name=f"xbf{d}") for d in range(DT)]
        vres = [vpool.tile([P, L], BF16, name=f"vres{d}") for d in range(DT)]

        def layernorm(which, lns_s, lnb_s):
            """xbf[dt] = ((vres - mu) * rstd) * s + b, stats along D."""
            # pass 1: sums
            sum_ps = psum.tile([1, L], F32, name="sum_ps", tag="row", bufs=1)
            for dt in range(DT):
                for (n0, nw) in NSL:
                    nc.tensor.matmul(
                        sum_ps[:, n0:n0 + nw], ones_col, vres[dt][:, n0:n0 + nw],
                        start=(dt == 0), stop=(dt == DT - 1), skip_group_check=True,
                    )
            m_row = work.tile([1, L], F32, name="m_row", tag="rows", bufs=3)
            nc.vector.tensor_scalar_mul(m_row, sum_ps, 1.0 / D)
            # pass 2: sum of squares (+ eps folded into the mean-square)
            sq_ps = psum.tile([1, L], F32, name="sq_ps", tag="row", bufs=1)
            for dt in range(DT):
                vsq = work.tile([P, L], BF16, name="vsq", tag="f32a", bufs=2)
                nc.vector.tensor_mul(vsq, vres[dt], vres[dt])
                for (n0, nw) in NSL:
                    nc.tensor.matmul(
                        sq_ps[:, n0:n0 + nw], ones_col, vsq[:, n0:n0 + nw],
                        start=(dt == 0), stop=(dt == DT - 1), skip_group_check=True,
                    )
            msq_row = work.tile([1, L], F32, name="msq_row", tag="rows", bufs=3)
            nc.vector.tensor_scalar(msq_row, sq_ps, 1.0 / D, EPS,
                                    op0=mybir.AluOpType.mult, op1=mybir.AluOpType.add)
            var_row = work.tile([1, L], F32, name="var_row", tag="rows", bufs=3)
            nc.vector.tensor_mul(var_row, m_row, m_row)
            nc.vector.tensor_sub(var_row, msq_row, var_row)  # E[x^2]+eps - mu^2
            std_row = work.tile([1, L], F32, name="std_row", tag="rows", bufs=3)
            nc.scalar.activation(std_row, var_row, AF.Sqrt)
            a_row = work.tile([1, L], F32, name="a_row", tag="rows", bufs=3)
            nc.vector.reciprocal(a_row, std_row)
            c_row = work.tile([1, L], F32, name="c_row", tag="rows", bufs=3)
            nc.vector.tensor_mul(c_row, m_row, a_row)
            nc.vector.tensor_scalar_mul(c_row, c_row, -1.0)
            # broadcast rstd and -mu*rstd to [128, L]
            a_ps = ps_mm("a_ps")
            c_ps = ps_mm("c_ps")
            for (n0, nw) in NSL:
                nc.tensor.matmul(a_ps[:, n0:n0 + nw], ones_row128,
                                 a_row[:, n0:n0 + nw], start=True, stop=True,
                                 skip_group_check=True)
                nc.tensor.matmul(c_ps[:, n0:n0 + nw], ones_row128,
                                 c_row[:, n0:n0 + nw], start=True, stop=True,
                                 skip_group_check=True)
            for dt in range(DT):
                t = work.tile([P, L], F32, name="lnt", tag="f32b", bufs=2)
                nc.vector.tensor_mul(t, vres[dt], a_ps)
                nc.vector.tensor_add(t, t, c_ps)
                nc.vector.tensor_scalar(
                    xbf[dt], t,
                    lns_s[:, which * DT + dt:which * DT + dt + 1],
                    lnb_s[:, which * DT + dt:which * DT + dt + 1],
                    op0=mybir.AluOpType.mult, op1=mybir.AluOpType.add,
                )

        # =========== embedding: gather + scale + pe, then transpose ===========
        x0bf = []
        for lt in range(LT):
            idx_t = work.tile([P, 1], I32, name="idx", tag="idx")
            nc.sync.dma_start(out=idx_t, in_=tok[lt])
            g = work.tile([P, D], F32, name="gath", tag="f32a", bufs=2)
            nc.gpsimd.indirect_dma_start(
                out=g,
                out_offset=None,
                in_=emb_full[:, :],
                in_offset=bass.IndirectOffsetOnAxis(ap=idx_t[:, :1], axis=0),
            )
            pe_t = work.tile([P, D], F32, name="pet", tag="f32b", bufs=2)
            nc.sync.dma_start(out=pe_t, in_=pe_d[lt])
            x0 = work.tile([P, D], F32, name="x0", tag="f32a", bufs=2)
            nc.vector.tensor_scalar_mul(x0, g, EMB_SCALE)
            xb = work.tile([P, D], BF16, name=f"x0bf{lt}", tag="x0bf", bufs=8)
            nc.vector.tensor_add(xb, x0, pe_t)
            x0bf.append(xb)
        for dt in range(DT):
            for lt in range(LT):
                tp = psum.tile([P, P], BF16, name="tp", tag="mm", bufs=2)
                nc.tensor.transpose(tp, x0bf[lt][:, dt * P:(dt + 1) * P], ident)
                nc.vector.tensor_copy(xbf[dt][:, lt * P:(lt + 1) * P], tp)

        # ================= layers =================
        for ly in range(nlayers):
            wq_s = wpool.tile([P, DT, HVS], BF16, name="wq_s", tag="wq")
            nc.sync.dma_start(out=wq_s, in_=wq_d[ly].rearrange("k p m -> p k m"))
            wk_s = wpool.tile([P, DT, HVS], BF16, name="wk_s", tag="wk")
            nc.sync.dma_start(out=wk_s, in_=wk_d[ly].rearrange("k p m -> p k m"))
            wv_s = wpool.tile([P, DT, HVS], BF16, name="wv_s", tag="wv")
            nc.sync.dma_start(out=wv_s, in_=wv_d[ly].rearrange("k p m -> p k m"))
            wo_s = wpool.tile([P, DT, P], BF16, name="wo_s", tag="wo")
            nc.sync.dma_start(out=wo_s, in_=wo_d[ly].rearrange("k p m -> p k m"))
            w1_s = wpool.tile([P, DT, FT, P], BF16, name="w1_s", tag="w1")
            nc.sync.dma_start(out=w1_s, in_=w1_d[ly].rearrange("k f p m -> p k f m"))
            w2_s = wpool.tile([P, FT, DT, P], BF16, name="w2_s", tag="w2")
            nc.sync.dma_start(out=w2_s, in_=w2_d[ly].rearrange("f k p m -> p f k m"))
            b1_s = wpool.tile([P, FT], F32, name="b1_s", tag="b1")
            nc.sync.dma_start(out=b1_s, in_=b1_d[ly].rearrange("f p o -> p (f o)"))
            b2_s = wpool.tile([P, DT], F32, name="b2_s", tag="b2")
            nc.sync.dma_start(out=b2_s, in_=b2_d[ly].rearrange("d p o -> p (d o)"))
            lns_s = wpool.tile([P, 2 * DT], F32, name="lns_s", tag="lns")
            nc.sync.dma_start(out=lns_s, in_=lns_d[ly].rearrange("i d p o -> p (i d o)"))
            lnb_s = wpool.tile([P, 2 * DT], F32, name="lnb_s", tag="lnb")
            nc.sync.dma_start(out=lnb_s, in_=lnb_d[ly].rearrange("i d p o -> p (i d o)"))

            # ---- QKV projections ----
            scope = nc.named_scope
            def proj(w_s, outname):
                ps = ps_mm("proj_ps")
                for kt in range(DT):
                    for (n0, nw) in NSL:
                        nc.tensor.matmul(
                            ps[:, n0:n0 + nw],
                            w_s[:, kt, :],
                            xbf[kt][:, n0:n0 + nw],
                            start=(kt == 0),
                            stop=(kt == DT - 1),
                            skip_group_check=True,
                        )
                ot = work.tile([P, L], BF16, name=outname, tag=outname,
                               bufs=(1 if outname == "vT" else 2))
                nc.vector.tensor_copy(ot, ps)
                return ot

            with scope(f"L{ly}_qkv"):
                qT = proj(wq_s, "qT")
                kT = proj(wk_s, "kT")
                vT = proj(wv_s, "vT")

            # V natural
            vnat = []
            for kt in range(LT):
                tp = psum.tile([P, P], BF16, name="vtp", tag="mm", bufs=2)
                nc.tensor.transpose(tp, vT[:, kt * P:(kt + 1) * P], ident)
                vn = work.tile([P, 130], BF16, name="vnat", tag="vnat", bufs=8)
                nc.vector.memset(vn[:, 64:65], 1.0)
                nc.vector.memset(vn[:, 129:130], 1.0)
                nc.vector.tensor_copy(vn[:, 0:64], tp[:, 0:64])
                nc.vector.tensor_copy(vn[:, 65:129], tp[:, 64:128])
                vnat.append(vn)

            # ---- attention per head ----
            oT = work.tile([P, L], BF16, name="oT", tag="oT", bufs=2)
            _sid = nc.enter_named_scope(f"L{ly}_attn", False)[0]
            for h in range(HC):
                hp = 64 * h
                ot_ps = psum.tile([P, L], F32, name="ot_ps", tag="acc", bufs=1)
                for kt in range(LT):
                    q0 = kt * P
                    st_ps = ps_mm("st_ps")
                    for (n0, nw) in _nslices(q0, L):
                        nc.tensor.matmul(
                            st_ps[:, n0:n0 + nw],
                            kT[hp:hp + 64, q0:q0 + P],
                            qT[hp:hp + 64, n0:n0 + nw],
                            start=True, stop=True, skip_group_check=True,
                        )
                    pt = work.tile([P, L], BF16, name="pt", tag="pt", bufs=3)
                    nc.scalar.activation(
                        pt[:, q0:L], st_ps[:, q0:L], AF.Exp, scale=SCORE_SCALE
                    )
                    nc.vector.tensor_mul(pt[:, q0:q0 + P], pt[:, q0:q0 + P], tri01)
                    for (n0, nw0) in NSL:
                        a = max(n0, q0)
                        w = n0 + nw0 - a
                        if w <= 0:
                            continue
                        last_kt = min(LT - 1, (n0 + nw0 - 1) // P)
                        nc.tensor.matmul(
                            ot_ps[0:65, a:a + w],
                            vnat[kt][:, 65 * h:65 * h + 65],
                            pt[:, a:a + w],
                            start=(kt == 0),
                            stop=(kt == last_kt),
                            skip_group_check=True,
                        )
                # normalize rows 1:65 by row 0 (softmax sums)
                s_row = work.tile([1, L], F32, name="s_row", tag="rows", bufs=3)
                nc.vector.tensor_copy(s_row, ot_ps[64:65, :])
                r_row = work.tile([1, L], F32, name="r_row", tag="rows", bufs=3)
                nc.vector.reciprocal(r_row, s_row)
                bc_ps = psum.tile([64, L], F32, name="bc_ps", tag="mm", bufs=2)
                for (n0, nw) in NSL:
                    nc.tensor.matmul(
                        bc_ps[:, n0:n0 + nw], ones_row64, r_row[:, n0:n0 + nw],
                        start=True, stop=True, skip_group_check=True,
                    )
                bc_sb = work.tile([64, L], F32, name="bc_sb", tag="f32a", bufs=2)
                nc.vector.tensor_copy(bc_sb, bc_ps)
                nc.vector.tensor_mul(oT[hp:hp + 64, :], ot_ps[0:64, :], bc_sb)

            nc.leave_named_scope(f"L{ly}_attn", _sid, False)
            _sid = nc.enter_named_scope(f"L{ly}_woar", False)[0]
            # ---- Wo + AllReduce (residual x/8 folded) ----
            cc_in = dram.tile([P, DT * L], BF16, name="cc_in", tag="cc_in")
            cc_out = dram.tile(
                [P, DT * L], BF16, name="cc_out", tag="cc_out", addr_space="Shared"
            )
            for dt in range(DT):
                wo_ps = ps_mm("wo_ps")
                for (n0, nw) in NSL:
                    nc.tensor.matmul(
                        wo_ps[:, n0:n0 + nw], wo_s[:, dt, :], oT[:, n0:n0 + nw],
                        start=True, stop=True, skip_group_check=True,
                    )
                t1 = work.tile([P, L], F32, name="t1", tag="f32a", bufs=2)
                nc.vector.tensor_scalar_mul(t1, xbf[dt], 1.0 / NC)
                t2 = work.tile([P, L], BF16, name="t2", tag="f32b", bufs=2)
                nc.vector.tensor_add(t2, t1, wo_ps)
                nc.sync.dma_start(out=cc_in[:, dt * L:(dt + 1) * L], in_=t2)
            nc.gpsimd.collective_compute(
                "AllReduce", mybir.AluOpType.add, replica_groups=rg,
                ins=[cc_in[:, :].opt()], outs=[cc_out[:, :].opt()],
            )
            for dt in range(DT):
                nc.sync.dma_start(out=vres[dt], in_=cc_out[:, dt * L:(dt + 1) * L])

            nc.leave_named_scope(f"L{ly}_woar", _sid, False)
            with scope(f"L{ly}_ln1"):
                layernorm(0, lns_s, lnb_s)

            _sid = nc.enter_named_scope(f"L{ly}_ffn", False)[0]
            # ---- FFN1 ----
            hT = []
            for mt in range(FT):
                h_ps = ps_mm("h_ps")
                for kt in range(DT):
                    for (n0, nw) in NSL:
                        nc.tensor.matmul(
                            h_ps[:, n0:n0 + nw], w1_s[:, kt, mt, :],
                            xbf[kt][:, n0:n0 + nw],
                            start=(kt == 0), stop=(kt == DT - 1),
                            skip_group_check=True,
                        )
                ht = work.tile([P, L], BF16, name="hT", tag=f"hT{mt}", bufs=1)
                nc.scalar.activation(ht, h_ps, AF.Relu, bias=b1_s[:, mt:mt + 1])
                hT.append(ht)

            # ---- FFN2 + AllReduce (residual x/8 and b2/8 folded) ----
            cc2_in = dram.tile([P, DT * L], BF16, name="cc2_in", tag="cc_in")
            cc2_out = dram.tile(
                [P, DT * L], BF16, name="cc2_out", tag="cc_out", addr_space="Shared"
            )
            for dt in range(DT):
                f_ps = ps_mm("f_ps")
                for kt in range(FT):
                    for (n0, nw) in NSL:
                        nc.tensor.matmul(
                            f_ps[:, n0:n0 + nw], w2_s[:, kt, dt, :],
                            hT[kt][:, n0:n0 + nw],
                            start=(kt == 0), stop=(kt == FT - 1),
                            skip_group_check=True,
                        )
                t1 = work.tile([P, L], F32, name="t1b", tag="f32a", bufs=2)
                nc.vector.tensor_scalar(
                    t1, xbf[dt], 1.0 / NC, b2_s[:, dt:dt + 1],
                    op0=mybir.AluOpType.mult, op1=mybir.AluOpType.add,
                )
                t2 = work.tile([P, L], BF16, name="t2b", tag="f32b", bufs=2)
                nc.vector.tensor_add(t2, t1, f_ps)
                nc.sync.dma_start(out=cc2_in[:, dt * L:(dt + 1) * L], in_=t2)
            nc.gpsimd.collective_compute(
                "AllReduce", mybir.AluOpType.add, replica_groups=rg,
                ins=[cc2_in[:, :].opt()], outs=[cc2_out[:, :].opt()],
            )
            for dt in range(DT):
                nc.sync.dma_start(out=vres[dt], in_=cc2_out[:, dt * L:(dt + 1) * L])

            nc.leave_named_scope(f"L{ly}_ffn", _sid, False)
            with scope(f"L{ly}_ln2"):
                layernorm(1, lns_s, lnb_s)

        _sid = nc.enter_named_scope("logits", False)[0]
        # ================= final logits =================
        for (v0, vw) in _nslices(0, VS):
            ets = []
            for kt in range(DT):
                et = work.tile([P, 512], BF16, name="et", tag="et", bufs=16)
                nc.sync.dma_start(out=et[:, 0:vw], in_=embt_d[kt][:, v0:v0 + vw])
                ets.append(et)
            for lt in range(LT):
                lg_ps = psum.tile([P, 512], F32, name="lg_ps", tag="mm", bufs=2)
                for kt in range(DT):
                    nc.tensor.matmul(
                        lg_ps[:, 0:vw],
                        xbf[kt][:, lt * P:(lt + 1) * P],
                        ets[kt][:, 0:vw],
                        start=(kt == 0), stop=False, skip_group_check=True,
                    )
                nc.tensor.matmul(
                    lg_ps[:, 0:vw], ones_rowP_bf, fb_s[:, v0:v0 + vw],
                    start=False, stop=True, skip_group_check=True,
                )
                lg = work.tile([P, 512], F32, name="lg", tag="lg", bufs=2)
                nc.vector.tensor_copy(lg[:, 0:vw], lg_ps[:, 0:vw])
                nc.sync.dma_start(
                    out=logits_d[lt * P:(lt + 1) * P, v0:v0 + vw], in_=lg[:, 0:vw]
                )
        nc.leave_named_scope("logits", _sid, False)

    nc.finalize()
    return nc


# ====================== host side ======================

def _pe_host():
    i = 2.0 * np.arange(D // 2, dtype=np.float32)
    base = np.float32(10000.0) ** (-i / np.float32(D))
    half = np.arange(L, dtype=np.float32)[:, None] * base[None, :]
    return np.stack([np.sin(half), np.cos(half)], axis=-1).reshape(L, D).astype(np.float32)


def prep_inputs(inputs, nlayers=NLAYERS):
    bf = ml_dtypes.bfloat16
    tokens = np.asarray(inputs["tokens"]).reshape(L).astype(np.int32)
    emb = np.ascontiguousarray(np.asarray(inputs["emb"], dtype=np.float32))
    tok_t = np.ascontiguousarray(tokens.reshape(LT, P, 1))
    pe_t = np.ascontiguousarray(_pe_host().reshape(LT, P, D))

    Wq = np.asarray(inputs["Wq"], dtype=np.float32)[:nlayers]
    Wk = np.asarray(inputs["Wk"], dtype=np.float32)[:nlayers]
    Wv = np.asarray(inputs["Wv"], dtype=np.float32)[:nlayers]
    Wo = np.asarray(inputs["Wo"], dtype=np.float32)[:nlayers]
    W1 = np.asarray(inputs["W1"], dtype=np.float32)[:nlayers]
    W2 = np.asarray(inputs["W2"], dtype=np.float32)[:nlayers]
    b1 = np.asarray(inputs["b1"], dtype=np.float32)[:nlayers]
    b2 = np.asarray(inputs["b2"], dtype=np.float32)[:nlayers]
    l1s = np.asarray(inputs["ln1_s"], dtype=np.float32)[:nlayers]
    l1b = np.asarray(inputs["ln1_b"], dtype=np.float32)[:nlayers]
    l2s = np.asarray(inputs["ln2_s"], dtype=np.float32)[:nlayers]
    l2b = np.asarray(inputs["ln2_b"], dtype=np.float32)[:nlayers]
    fbv = np.asarray(inputs["final_b"], dtype=np.float32)

    def C(a, dtype=None):
        a = np.ascontiguousarray(a)
        return a.astype(dtype) if dtype is not None else a

    maps = []
    for c in range(NC):
        hs = slice(HC * c, HC * (c + 1))
        wq = Wq[:, hs].transpose(0, 2, 1, 3).reshape(nlayers, D, HVS)
        wk = Wk[:, hs].transpose(0, 2, 1, 3).reshape(nlayers, D, HVS)
        wv = Wv[:, hs].transpose(0, 2, 1, 3).reshape(nlayers, D, HVS)
        wo = Wo[:, P * c:P * (c + 1), :]        # [nl, 128(HV), D]
        w1 = W1[:, :, FS * c:FS * (c + 1)]      # [nl, D, FS]
        w2 = W2[:, FS * c:FS * (c + 1), :]      # [nl, FS, D]
        m = {
            "emb_full": emb,
            "tok": tok_t,
            "pe": pe_t,
            "wq": C(wq.reshape(nlayers, DT, P, HVS), bf),
            "wk": C(wk.reshape(nlayers, DT, P, HVS), bf),
            "wv": C(wv.reshape(nlayers, DT, P, HVS), bf),
            # wo_d[ly, dt] = Wo_c[:, 128dt:128dt+128]  ([HV=128, 128])
            "wo": C(wo.reshape(nlayers, P, DT, P).transpose(0, 2, 1, 3), bf),
            # w1_d[ly, kt, mt] = W1_c[128kt:.., 128mt:..]
            "w1": C(w1.reshape(nlayers, DT, P, FT, P).transpose(0, 1, 3, 2, 4), bf),
            # w2_d[ly, kt, dt] = W2_c[128kt:.., 128dt:..]
            "w2": C(w2.reshape(nlayers, FT, P, DT, P).transpose(0, 1, 3, 2, 4), bf),
            "b1c": C(b1[:, FS * c:FS * (c + 1)].reshape(nlayers, FT, P, 1)),
            "b2c": C((b2 / NC).reshape(nlayers, DT, P, 1)),
            "lns": C(np.stack([l1s, l2s], axis=1).reshape(nlayers, 2, DT, P, 1)),
            "lnb": C(np.stack([l1b, l2b], axis=1).reshape(nlayers, 2, DT, P, 1)),
            "embt": C(emb[VS * c:VS * (c + 1), :].T.reshape(DT, P, VS), bf),
            "fb": C(fbv[VS * c:VS * (c + 1)].reshape(1, VS), bf),
        }
        maps.append(m)
    return maps


_PROGRAM = None


def kernel(**inputs):
    global _PROGRAM
    if _PROGRAM is None:
        _PROGRAM = build_program(NLAYERS)
    maps = prep_inputs(inputs, NLAYERS)
    res = run_bass_kernel_spmd(_PROGRAM, maps, core_ids=list(range(NC)), trace=False)
    logits = np.concatenate(
        [res.results[c]["logits"] for c in range(NC)], axis=1
    ).astype(np.float32)
    return logits.reshape(1, L, V)


if __name__ == "__main__":
    nc = build_program(int(sys.argv[1]) if len(sys.argv) > 1 else 1)
    print("build OK; instructions:", len(nc.inst_map))
